# revision 1
# baseline (speedup 1.0000x reference)
"""Fused OOQKV attention-with-generated-transform kernel for Trainium2.

Math (per head h):
  g = gelu(x @ Wg_h + bg_h)            # [T, 64, 64] per-token transform
  q,k,v = x @ W{q,k,v}_h + b           # [T, 64]
  qg[t] = q[t] @ g[t]
  att = softmax(qg @ k^T)              # per batch, no scaling
  out_h = att @ v

Sharding: head-parallel, 1 head per core (8 heads, 8 cores); every core
reads the full (host-pre-transposed) xT.

Per-core schedule:
  phase 1 (per 128-token tile): fused q|v_aug|k projection and the
    32768-wide g projection, grouped so consecutive PE matmuls share the
    stationary xT slice (f32r weight switches cost ~2x); biases are K=1
    bf16 matmuls (bf16 keeps them at stream rate; bias magnitudes are
    ~0.04 so bf16 rounding is ~1e-4 absolute). ACT applies exact gelu,
    writing each 512-chunk transposed to (e-major, d-minor) layout so the
    DVE qg contraction multiplies contiguously against a broadcast q view
    and reduces over a contiguous innermost d. PE transposes build kT and
    qgT for phase 2.
  phase 2 (per batch, per 512 query cols): S^T = kT-slice.T @ qgT on PE,
    exp on ACT (no max subtraction; |scores| < 70 so fp32 exp is exact
    enough), then out^T accumulated over m-tiles with v augmented by a
    ones column so row 64 carries the softmax denominator.
Host divides by the denominator row and transposes during the gather.

Matmuls run in float32r (fp32-reduced: 1 cycle/row streaming, ~1e-4
matmul rel err measured on HW); end-to-end rel err vs the fp32 reference
is ~1e-3.
"""

import sys

sys.path.insert(0, "/opt/trn_rl_repo")

import numpy as np

B, N, E, H, D = 4, 1024, 512, 8, 64
T = B * N                 # 4096 flattened tokens
OC = 512                  # g-matmul output chunk
NOC = (D * D) // OC       # 8 chunks per head
DPC = OC // D             # 8 d-values per chunk
NTT = T // 128            # 32 token tiles
NKT = E // 128            # 4 contraction tiles
QVKW = 256                # fused q|v_aug|k projection width (zero padded)
M = 8                     # cores

_cache = {}


def _build():
    if "nc" in _cache:
        return _cache["nc"]
    from contextlib import ExitStack

    import concourse.bass as bass
    import concourse.bacc as bacc
    import concourse.mybir as mybir
    import concourse.tile as tile
    from concourse.masks import make_identity

    F32 = mybir.dt.float32
    F32R = mybir.dt.float32r
    BF16 = mybir.dt.bfloat16
    AF = mybir.ActivationFunctionType
    ALU = mybir.AluOpType
    AX = mybir.AxisListType

    nc = bacc.Bacc(trn_type="TRN2")
    xT_d = nc.dram_tensor("xT", [E, T], F32R, kind="ExternalInput")
    Wg_d = nc.dram_tensor("Wg", [E, D * D], F32R, kind="ExternalInput")
    bg_d = nc.dram_tensor("bg", [1, D * D], BF16, kind="ExternalInput")
    Wqvk_d = nc.dram_tensor("Wqvk", [E, QVKW], F32R, kind="ExternalInput")
    bqvk_d = nc.dram_tensor("bqvk", [1, QVKW], BF16, kind="ExternalInput")
    outT_d = nc.dram_tensor("outT", [D + 1, T], F32, kind="ExternalOutput")

    with tile.TileContext(nc) as tc, ExitStack() as ctx:
        const = ctx.enter_context(tc.tile_pool(name="const", bufs=1))
        acts = ctx.enter_context(tc.tile_pool(name="acts", bufs=1))

        wqvk_sb = []
        for kt in range(NKT):
            wqt = const.tile([128, QVKW], F32R, tag=f"wqvk{kt}")
            nc.sync.dma_start(wqt[:], Wqvk_d[kt * 128:(kt + 1) * 128, :])
            wqvk_sb.append(wqt)
        bg_sb = const.tile([1, D * D], BF16)
        nc.sync.dma_start(bg_sb[:], bg_d[:, :])
        bqvk_sb = const.tile([1, QVKW], BF16)
        nc.sync.dma_start(bqvk_sb[:], bqvk_d[:, :])
        ones32 = const.tile([1, 128], F32)
        nc.gpsimd.memset(ones32[:], 1.0)
        ones_b = const.tile([1, 128], BF16)
        nc.gpsimd.tensor_copy(ones_b[:], ones32[:])
        ident = const.tile([128, 128], F32)
        make_identity(nc, ident[:])

        # persistent per-head activations
        q_sb = acts.tile([128, NTT, D], F32)       # q, natural layout
        v_sb = acts.tile([128, NTT, D + 1], F32R)  # v | ones column
        kT_sb = acts.tile([D, T], F32R)
        qgT_sb = acts.tile([D, T], F32R)

        # ---------------- phase 1: projections, g, qg ----------------
        with ExitStack() as p1:
            xpool = p1.enter_context(tc.tile_pool(name="xp", bufs=2))
            wgpool = p1.enter_context(tc.tile_pool(name="wgp", bufs=1))
            wg_sb = []
            for kt in range(NKT):
                wgt = wgpool.tile([128, D * D], F32R, tag=f"wg{kt}",
                                  name=f"wg{kt}")
                wg_sb.append(wgt)
            QL = (D * D) // 4
            for quar in range(4):
                for kt in range(NKT):
                    nc.scalar.dma_start(
                        wg_sb[kt][:, quar * QL:(quar + 1) * QL],
                        Wg_d[kt * 128:(kt + 1) * 128,
                             quar * QL:(quar + 1) * QL])
            gpool = p1.enter_context(tc.tile_pool(name="gp", bufs=5))
            dpool = p1.enter_context(tc.tile_pool(name="dp", bufs=4))
            pp_g = p1.enter_context(
                tc.tile_pool(name="pg", bufs=7, space="PSUM"))
            pp_qvk = pp_g
            pp_tr = p1.enter_context(
                tc.tile_pool(name="ptr", bufs=1, space="PSUM"))

            pending = []  # (tc0, k_nat, qg_t) awaiting PE transpose

            def flush_pending():
                for ptc0, pk, pqg in pending:
                    ptr = pp_tr.tile([D, 128], F32, tag="tr", name="ktr")
                    nc.tensor.transpose(ptr[:], pk[:], ident[:])
                    nc.vector.tensor_copy(kT_sb[:, ptc0:ptc0 + 128], ptr[:])
                    ptr2 = pp_tr.tile([D, 128], F32, tag="tr", name="qgtr")
                    nc.tensor.transpose(ptr2[:], pqg[:], ident[:])
                    nc.vector.tensor_copy(qgT_sb[:, ptc0:ptc0 + 128], ptr2[:])
                pending.clear()

            for tt in range(NTT):
                tc0 = tt * 128
                xs = []
                for kt in range(NKT):
                    xt = xpool.tile([128, 128], F32R, tag=f"x{kt}")
                    nc.sync.dma_start(
                        xt[:], xT_d[kt * 128:(kt + 1) * 128, tc0:tc0 + 128])
                    xs.append(xt)

                # two rounds of 4 g-chunks; round 0 also carries the qvk
                # projection so each (round, kt) is a same-lhsT matmul run
                pq = pp_qvk.tile([128, OC], F32, tag="pg", name="pq")
                pgs = {}
                for rnd in range(2):
                    for kt in range(NKT):
                        if rnd == 0:
                            nc.tensor.matmul(pq[:, 0:QVKW], xs[kt][:],
                                             wqvk_sb[kt][:],
                                             start=(kt == 0), stop=False)
                        for oc in range(rnd * 4, rnd * 4 + 4):
                            oc0 = oc * OC
                            if kt == 0:
                                pgs[oc] = pp_g.tile([128, OC], F32, tag="pg", name=f"pg{oc}")
                            nc.tensor.matmul(
                                pgs[oc][:], xs[kt][:],
                                wg_sb[kt][:, oc0:oc0 + OC],
                                start=(kt == 0), stop=False)
                    if rnd == 0:
                        nc.tensor.matmul(pq[:, 0:QVKW], ones_b[:],
                                         bqvk_sb[:], start=False, stop=True)
                    for oc in range(rnd * 4, rnd * 4 + 4):
                        oc0 = oc * OC
                        nc.tensor.matmul(pgs[oc][:], ones_b[:],
                                         bg_sb[:, oc0:oc0 + OC],
                                         start=False, stop=True)
                    if rnd == 0:
                        flush_pending()  # prior tile's transposes mid-stream

                nc.vector.tensor_copy(q_sb[:, tt, :], pq[:, 0:D])
                nc.vector.tensor_copy(v_sb[:, tt, :], pq[:, D:2 * D + 1])
                k_nat = dpool.tile([128, D], F32, tag="knat")
                nc.vector.tensor_copy(k_nat[:], pq[:, 2 * D + 1:3 * D + 1])

                # gelu + qg contraction per chunk
                qg_part = dpool.tile([128, NOC, D], F32, tag="qgp")
                for oc in range(NOC):
                    # gelu, written (e-major, d-minor) so the d-reduce is
                    # contiguous
                    g_t = gpool.tile([128, OC], F32, tag="g")
                    gw = g_t[:]
                    g_ed = bass.AP(tensor=gw.tensor, offset=gw.offset,
                                   ap=[gw.ap[0], [1, DPC], [DPC, D]])
                    nc.scalar.activation(g_ed, pgs[oc][:], AF.Gelu)
                    prod = gpool.tile([128, OC], F32, tag="prod")
                    qs = q_sb[:, tt, :]
                    q3 = bass.AP(
                        tensor=qs.tensor,
                        offset=qs.offset + oc * DPC,
                        ap=[qs.ap[0], [0, D], [1, DPC]])
                    nc.vector.tensor_tensor(
                        prod[:].rearrange("p (e d) -> p e d", d=DPC),
                        g_t[:].rearrange("p (e d) -> p e d", d=DPC),
                        q3, op=ALU.mult)
                    nc.vector.tensor_reduce(
                        qg_part[:, oc, :],
                        prod[:].rearrange("p (e d) -> p e d", d=DPC),
                        axis=AX.X, op=ALU.add)
                qg_t = dpool.tile([128, D], F32, tag="qg")
                qp = qg_part[:]
                qpv = bass.AP(tensor=qp.tensor, offset=qp.offset,
                              ap=[qp.ap[0], [1, D], [D, NOC]])
                nc.vector.tensor_reduce(qg_t[:], qpv, axis=AX.X, op=ALU.add)
                pending.append((tc0, k_nat, qg_t))
            flush_pending()

        # ---------------- phase 2: attention ----------------
        with ExitStack() as p2:
            espool = p2.enter_context(tc.tile_pool(name="es", bufs=34))
            outp = p2.enter_context(tc.tile_pool(name="outp", bufs=4))
            pp_s = p2.enter_context(
                tc.tile_pool(name="psc", bufs=6, space="PSUM"))
            pp_av = p2.enter_context(
                tc.tile_pool(name="pav", bufs=2, space="PSUM"))

            NMT = N // 128  # m tiles per batch
            NNC = N // OC   # n chunks per batch
            pending_av = []  # (b, nch, es-dict) awaiting av emission

            def emit_av():
                if not pending_av:
                    return
                by_b = {}
                for bb, nch, esd in pending_av:
                    by_b.setdefault(bb, {})[nch] = esd
                for bb, chunks in by_b.items():
                    pavs = {nch: pp_av.tile([D + 1, OC], F32, tag="av",
                                            name=f"pav{nch}")
                            for nch in chunks}
                    for mt in range(NMT):
                        for nch, esd in chunks.items():
                            nc.tensor.matmul(pavs[nch][:],
                                             v_sb[:, bb * NMT + mt, :],
                                             esd[mt][:],
                                             start=(mt == 0),
                                             stop=(mt == NMT - 1))
                    for nch in chunks:
                        nc0 = bb * N + nch * OC
                        o_t = outp.tile([D + 1, OC], F32, tag="o", name="o_t")
                        nc.vector.tensor_copy(o_t[:], pavs[nch][:])
                        nc.sync.dma_start(outT_d[:, nc0:nc0 + OC], o_t[:])
                pending_av.clear()

            for b in range(B):
                es = {}
                # S^T and exp for the whole batch; kT slice (lhsT) is
                # reused across both n-chunks
                for mt in range(NMT):
                    if mt == 2:
                        emit_av()  # prior batch's av, mid-stream
                    mc0 = b * N + mt * 128
                    for nch in range(NNC):
                        nc0 = b * N + nch * OC
                        ps_ = pp_s.tile([128, OC], F32, tag="s")
                        nc.tensor.matmul(ps_[:], kT_sb[:, mc0:mc0 + 128],
                                         qgT_sb[:, nc0:nc0 + OC],
                                         start=True, stop=True)
                        e_t = espool.tile([128, OC], F32R, tag="es")
                        nc.scalar.activation(e_t[:], ps_[:], AF.Exp)
                        es[(mt, nch)] = e_t
                for nch in range(NNC):
                    pending_av.append((b, nch, {mt: es[(mt, nch)]
                                                for mt in range(NMT)}))
            emit_av()

    nc.compile()
    _cache["nc"] = nc
    return nc


def _make_in_maps(x, Wq, bq, Wk, bk, Wv, bv, Wg, bg):
    import ml_dtypes
    x = np.asarray(x, dtype=np.float32)
    xT = np.ascontiguousarray(x.reshape(T, E).T)
    in_maps = []
    for h in range(M):
        c0 = h * D
        Wqvk = np.zeros((E, QVKW), dtype=np.float32)
        Wqvk[:, 0:D] = Wq[:, c0:c0 + D]
        Wqvk[:, D:2 * D] = Wv[:, c0:c0 + D]
        # column 2*D is the ones column of v_aug: weight 0, bias 1
        Wqvk[:, 2 * D + 1:3 * D + 1] = Wk[:, c0:c0 + D]
        bqvk = np.zeros((1, QVKW), dtype=np.float32)
        bqvk[0, 0:D] = bq[c0:c0 + D]
        bqvk[0, D:2 * D] = bv[c0:c0 + D]
        bqvk[0, 2 * D] = 1.0
        bqvk[0, 2 * D + 1:3 * D + 1] = bk[c0:c0 + D]
        g0 = h * D * D
        in_maps.append(dict(
            xT=xT,
            Wg=np.ascontiguousarray(Wg[:, g0:g0 + D * D], dtype=np.float32),
            bg=np.ascontiguousarray(bg[g0:g0 + D * D], dtype=np.float32)
            .reshape(1, D * D).astype(ml_dtypes.bfloat16),
            Wqvk=Wqvk,
            bqvk=bqvk.astype(ml_dtypes.bfloat16),
        ))
    return in_maps


def kernel(x, Wq, bq, Wk, bk, Wv, bv, Wg, bg):
    from concourse import bass_utils

    nc = _build()
    in_maps = _make_in_maps(x, Wq, bq, Wk, bk, Wv, bv, Wg, bg)
    res = bass_utils.run_bass_kernel_spmd(nc, in_maps, core_ids=list(range(M)))
    out = np.empty((B, N, H, D), dtype=np.float32)
    for h in range(M):
        oT = res.results[h]["outT"]           # [65, T]
        o = (oT[:D] / oT[D:D + 1]).T          # [T, 64]
        out[:, :, h, :] = o.reshape(B, N, D)
    return out.reshape(B, N, E)



# revision 15
# speedup vs baseline: 1.1387x; 1.1387x over previous
"""Fused OOQKV attention-with-generated-transform kernel for Trainium2.

Math (per head h):
  g = gelu(x @ Wg_h + bg_h)            # [T, 64, 64] per-token transform
  q,k,v = x @ W{q,k,v}_h + b           # [T, 64]
  qg[t] = q[t] @ g[t]
  att = softmax(qg @ k^T)              # per batch, no scaling
  out_h = att @ v

Sharding: head-parallel, 1 head per core (8 heads, 8 cores).

Implementation notes (v2):
- All projections run as error-compensated fp8e4m3 DoubleRow matmuls
  (0.5 cycles/row, K=256 per instruction): preact*sx*sw =
  x8@W8 + x8@dW8 + dx8'@W8d, where x8=f8(x*sx), dx8=f8(x*sx-x8),
  W8=f8(W*sw), dW8=f8(W*sw-W8). The final 1/(sx*sw) scale rides on the
  gelu ACT instruction. Row 511 of dx8' is hijacked to a constant ALPHA
  and row 511 of W8d carries f8(bias*sx*sw/ALPHA), folding the bias into
  the correction pass for free (costs one contraction row's correction).
- Wg columns are host-permuted e-major (col' = e*64+d) so gelu writes
  f16 contiguously and the qg d-reduction is innermost.
- qg on DVE: one f16 tensor_tensor multiply (2x_1p mode) against a
  broadcast q view, then a binary tree of f16 adds (all 2x) down to d=8
  and a final tensor_reduce; tensor_reduce has no DVE fast mode so the
  tree keeps most of the reduction at 2 elem/cycle.
- PSUM: one 5-bank pool shared by qvk/g/S tiles (tag rotation), 1 bank
  for PE transposes, 2 for AV accumulation.
- Phase 2 (S=kT.T@qgT per 128x512 tile, exp on ACT, AV=v.T@es) is
  interleaved into the next batch's token tiles to keep all engines fed.
- Small PSUM->SBUF copies run on GpSimd, which is otherwise idle.
Host divides by the softmax denominator row (v is augmented with a ones
column memset on-chip) and transposes during the gather.
"""

import sys

sys.path.insert(0, "/opt/trn_rl_repo")

import numpy as np

B, N, E, H, D = 4, 1024, 512, 8, 64
T = B * N                 # 4096 flattened tokens
OC = 512                  # g output chunk (one PSUM bank)
NOC = (D * D) // OC       # 8 chunks per head
NTT = T // 128            # 32 token tiles
QVKW = 192                # fused q|v|k projection width
M = 8                     # cores
NMT = N // 128            # m tiles per batch
NNC = N // OC             # n chunks per batch

SX = 32.0                 # x fp8 scale
SW = 4096.0               # weight fp8 scale
ALPHA = 128.0             # bias-row driver value in dx8'
ISCALE = 1.0 / (SX * SW)

_cache = {}


def _build():
    if "nc" in _cache:
        return _cache["nc"]
    from contextlib import ExitStack

    import concourse.bass as bass
    import concourse.bacc as bacc
    import concourse.mybir as mybir
    import concourse.tile as tile
    from concourse.masks import make_identity

    F32 = mybir.dt.float32
    F32R = mybir.dt.float32r
    F16 = mybir.dt.float16
    F8 = mybir.dt.float8e4
    AF = mybir.ActivationFunctionType
    ALU = mybir.AluOpType
    AX = mybir.AxisListType
    DR = mybir.MatmulPerfMode.DoubleRow

    nc = bacc.Bacc(trn_type="TRN2")

    # fp8 operands, DoubleRow layout: [p, s*COLS + c] = val(k=kp*256+s*128+p, c)
    xg_d = {}
    for nm in ("x8a", "x8b", "dx8a", "dx8b"):
        xg_d[nm] = nc.dram_tensor(nm, [128, 2 * T], F8, kind="ExternalInput")
    wg_d = {}
    for nm in ("W8a", "W8b", "dW8a", "dW8b", "W8da", "W8db"):
        wg_d[nm] = nc.dram_tensor(nm, [128, 2 * D * D], F8, kind="ExternalInput")
    wq_d = {}
    for nm in ("Wq8a", "Wq8b", "dWq8a", "dWq8b", "Wq8da", "Wq8db"):
        wq_d[nm] = nc.dram_tensor(nm, [128, 2 * QVKW], F8, kind="ExternalInput")
    outT_d = nc.dram_tensor("outT", [D + 1, T], F32, kind="ExternalOutput")

    def dr3(t, width, col0, ncols):
        """3-D DoubleRow view [128, 2, ncols] of a [128, 2*width] tile."""
        w = t[:]
        return bass.AP(tensor=w.tensor, offset=w.offset + col0,
                       ap=[w.ap[0], [width, 2], [1, ncols]])

    with tile.TileContext(nc) as tc, ExitStack() as ctx:
        const = ctx.enter_context(tc.tile_pool(name="const", bufs=1))
        acts = ctx.enter_context(tc.tile_pool(name="acts", bufs=1))

        # constants / persistent weights
        x_sb = {}
        for nm in ("x8a", "x8b", "dx8a", "dx8b"):
            t = const.tile([128, 2 * T], F8, tag=nm)
            nc.sync.dma_start(t[:], xg_d[nm][:, :])
            x_sb[nm] = t
        w_sb = {}
        for nm in ("W8a", "W8b", "dW8a", "dW8b", "W8da", "W8db"):
            t = const.tile([128, 2 * D * D], F8, tag=nm)
            nc.scalar.dma_start(t[:], wg_d[nm][:, :])
            w_sb[nm] = t
        wq_sb = {}
        for nm in ("Wq8a", "Wq8b", "dWq8a", "dWq8b", "Wq8da", "Wq8db"):
            t = const.tile([128, 2 * QVKW], F8, tag=nm)
            nc.scalar.dma_start(t[:], wq_d[nm][:, :])
            wq_sb[nm] = t
        ident = const.tile([128, 128], F16)
        make_identity(nc, ident[:])

        # persistent per-head activations
        v_sb = acts.tile([128, NTT, D + 1], F32R)   # v | ones column
        ones32 = const.tile([128, 1], F32)
        nc.gpsimd.memset(ones32[:], 1.0)
        ov = ones32[:]
        ones_b = bass.AP(tensor=ov.tensor, offset=ov.offset,
                         ap=[ov.ap[0], [0, NTT]])
        vv = v_sb[:]
        vcol = bass.AP(tensor=vv.tensor, offset=vv.offset + D,
                       ap=[vv.ap[0], [D + 1, NTT]])
        nc.vector.tensor_copy(vcol, ones_b)
        kT_sb = acts.tile([D, T], F16)
        qgT_sb = acts.tile([D, T], F16)

        pmain = ctx.enter_context(
            tc.tile_pool(name="pmain", bufs=5, space="PSUM"))
        ptr = ctx.enter_context(
            tc.tile_pool(name="ptr", bufs=1, space="PSUM"))
        pav = ctx.enter_context(
            tc.tile_pool(name="pav", bufs=2, space="PSUM"))

        gpool = ctx.enter_context(tc.tile_pool(name="gp", bufs=2))
        dpool = ctx.enter_context(tc.tile_pool(name="dp", bufs=2))
        tpool = ctx.enter_context(tc.tile_pool(name="tp", bufs=1))
        espool = ctx.enter_context(tc.tile_pool(name="es", bufs=17))
        outp = ctx.enter_context(tc.tile_pool(name="outp", bufs=2))

        es = {}           # (b, mt, nch) -> es tile
        pav_live = {}     # nch -> pav tile for the batch being drained

        S_SCHED = {1: [(0, 0), (1, 0), (2, 0)], 2: [(3, 0), (4, 0), (5, 0)],
                   3: [(6, 0), (7, 0)], 4: [(0, 1), (1, 1)],
                   5: [(2, 1), (3, 1), (4, 1)], 6: [(5, 1), (6, 1)],
                   7: [(7, 1)]}

        def emit_phase2(b, step):
            """Emit one slice of batch b's attention during a later tt."""
            for mt, nch in S_SCHED.get(step, []):
                mc0 = b * N + mt * 128
                nc0 = b * N + nch * OC
                ps_ = pmain.tile([128, OC], F32, tag="pg", name="ps")
                nc.tensor.matmul(ps_[:], kT_sb[:, mc0:mc0 + 128],
                                 qgT_sb[:, nc0:nc0 + OC],
                                 start=True, stop=True)
                e_t = espool.tile([128, OC], F32R, tag="es")
                nc.scalar.activation(e_t[:], ps_[:], AF.Exp)
                es[(b, mt, nch)] = e_t

            def av_half(nch, lo):
                if lo == 0:
                    pav_live[nch] = pav.tile([D + 1, OC], F32, tag="av",
                                             name=f"pav{nch}")
                for mt in range(lo, lo + 4):
                    nc.tensor.matmul(pav_live[nch][:],
                                     v_sb[:, b * NMT + mt, :],
                                     es[(b, mt, nch)][:],
                                     start=(mt == 0), stop=(mt == 7))

            def drain(nch):
                nc0 = b * N + nch * OC
                o_t = outp.tile([D + 1, OC], F32, tag="o", name="o_t")
                nc.vector.tensor_copy(o_t[:], pav_live[nch][:])
                nc.sync.dma_start(outT_d[:, nc0:nc0 + OC], o_t[:])

            if step == 5:
                av_half(0, 0)
            elif step == 6:
                av_half(0, 4)
                drain(0)
                av_half(1, 0)
            elif step == 7:
                av_half(1, 4)
                drain(1)

        prev_tr = []      # (tc0, k_nat, qg_t) awaiting PE transpose

        def emit_transposes():
            for tc0, kn, qt in prev_tr:
                for src, dst in ((kn, kT_sb), (qt, qgT_sb)):
                    p_t = ptr.tile([D, 128], F16, tag="tr", name="tr")
                    nc.tensor.transpose(p_t[:], src[:], ident[:])
                    nc.vector.tensor_copy(dst[:, tc0:tc0 + 128], p_t[:])
            prev_tr.clear()

        for tt in range(NTT):
            tc0 = tt * 128
            b_prev = tt // 8 - 1
            step = tt % 8
            if b_prev >= 0:
                emit_phase2(b_prev, step)

            xa = dr3(x_sb["x8a"], T, tc0, 128)
            xb = dr3(x_sb["x8b"], T, tc0, 128)
            dxa = dr3(x_sb["dx8a"], T, tc0, 128)
            dxb = dr3(x_sb["dx8b"], T, tc0, 128)

            g_buf = gpool.tile([128, NOC * OC], F16, tag="g")
            pq = None
            pgs = {}

            # chunk pairs: (qvk, c0), (c1,c2), (c3,c4), (c5,c6), (c7,)
            pairs = [("qvk", 0), (1, 2), (3, 4), (5, 6), (7,)]
            for pair in pairs:
                members = []
                for m_ in pair:
                    if m_ == "qvk":
                        pq = pmain.tile([128, OC], F32, tag="pg", name="pq")
                        members.append(("qvk", pq))
                    else:
                        pgs[m_] = pmain.tile([128, OC], F32, tag="pg",
                                             name=f"pg{m_}")
                        members.append((m_, pgs[m_]))

                def rhs_of(m_, wnm, qnm):
                    if m_ == "qvk":
                        return dr3(wq_sb[qnm], QVKW, 0, QVKW)
                    return dr3(w_sb[wnm], D * D, m_ * OC, OC)

                def out_of(m_, pt):
                    return pt[:, 0:QVKW] if m_ == "qvk" else pt[:]

                # pass structure: x8a{W,dW}, x8b{W,dW}, dx8a{W8d}, dx8b{W8d}
                for m_, pt in members:
                    o = out_of(m_, pt)
                    nc.tensor.matmul(o, xa, rhs_of(m_, "W8a", "Wq8a"),
                                     start=True, stop=False, perf_mode=DR)
                    nc.tensor.matmul(o, xa, rhs_of(m_, "dW8a", "dWq8a"),
                                     start=False, stop=False, perf_mode=DR)
                for m_, pt in members:
                    o = out_of(m_, pt)
                    nc.tensor.matmul(o, xb, rhs_of(m_, "W8b", "Wq8b"),
                                     start=False, stop=False, perf_mode=DR)
                    nc.tensor.matmul(o, xb, rhs_of(m_, "dW8b", "dWq8b"),
                                     start=False, stop=False, perf_mode=DR)
                for m_, pt in members:
                    nc.tensor.matmul(out_of(m_, pt), dxa,
                                     rhs_of(m_, "W8da", "Wq8da"),
                                     start=False, stop=False, perf_mode=DR)
                for m_, pt in members:
                    nc.tensor.matmul(out_of(m_, pt), dxb,
                                     rhs_of(m_, "W8db", "Wq8db"),
                                     start=False, stop=True, perf_mode=DR)

                # drains for this pair; q/k rescaled to true units during
                # the f16 copy, v kept raw-scaled (host folds ISCALE after
                # the softmax division -- the denominator is unscaled)
                for m_, pt in members:
                    if m_ == "qvk":
                        q_t = dpool.tile([128, D], F16, tag="q")
                        nc.vector.tensor_scalar_mul(q_t[:], pt[:, 0:D],
                                                    ISCALE)
                        k_nat = dpool.tile([128, D], F16, tag="k")
                        nc.vector.tensor_scalar_mul(k_nat[:],
                                                    pt[:, 2 * D:3 * D],
                                                    ISCALE)
                        nc.vector.tensor_copy(v_sb[:, tt, 0:D],
                                              pt[:, D:2 * D])
                    else:
                        nc.scalar.activation(
                            g_buf[:, m_ * OC:(m_ + 1) * OC], pt[:],
                            AF.Gelu, scale=ISCALE)

            # qg contraction on DVE (all f16, 2x mode)
            prod = dpool.tile([128, NOC * OC], F16, tag="prod")
            gv = g_buf[:]
            g3 = bass.AP(tensor=gv.tensor, offset=gv.offset,
                         ap=[gv.ap[0], [D, D], [1, D]])
            qv = q_t[:]
            q3 = bass.AP(tensor=qv.tensor, offset=qv.offset,
                         ap=[qv.ap[0], [0, D], [1, D]])
            pv = prod[:]
            p3 = bass.AP(tensor=pv.tensor, offset=pv.offset,
                         ap=[pv.ap[0], [D, D], [1, D]])
            nc.vector.tensor_tensor(p3, g3, q3, op=ALU.mult)

            tre1 = tpool.tile([128, D * 32], F16, tag="t1")
            tre2 = tpool.tile([128, D * 16], F16, tag="t2")
            tre3 = tpool.tile([128, D * 8], F16, tag="t3")
            lvl_in, w_ = prod, 64
            for tre in (tre1, tre2, tre3):
                h_ = w_ // 2
                iv = lvl_in[:]
                a0 = bass.AP(tensor=iv.tensor, offset=iv.offset,
                             ap=[iv.ap[0], [w_, D], [1, h_]])
                a1 = bass.AP(tensor=iv.tensor, offset=iv.offset + h_,
                             ap=[iv.ap[0], [w_, D], [1, h_]])
                ov = tre[:]
                o3 = bass.AP(tensor=ov.tensor, offset=ov.offset,
                             ap=[ov.ap[0], [h_, D], [1, h_]])
                nc.vector.tensor_tensor(o3, a0, a1, op=ALU.add)
                lvl_in, w_ = tre, h_
            qg_t = dpool.tile([128, D], F16, tag="qg")
            t3v = tre3[:]
            t33 = bass.AP(tensor=t3v.tensor, offset=t3v.offset,
                          ap=[t3v.ap[0], [8, D], [1, 8]])
            with nc.allow_low_precision(reason="f16 qg partials"):
                nc.vector.tensor_reduce(qg_t[:], t33, axis=AX.X, op=ALU.add)

            emit_transposes()
            prev_tr.append((tc0, k_nat, qg_t))
        emit_transposes()
        for step in range(1, 8):
            emit_phase2(B - 1, step)

    nc.compile()
    _cache["nc"] = nc
    return nc


def _f8(a):
    import ml_dtypes
    a = np.asarray(a, np.float32)
    assert np.abs(a).max() < 230.0, np.abs(a).max()
    return a.astype(ml_dtypes.float8_e4m3)


def _dr_pack(m, ncols):
    """[512, C] f8 array -> two [128, 2*C] DoubleRow-layout arrays."""
    out = []
    for kp in range(2):
        r = np.empty((128, 2 * ncols), dtype=m.dtype)
        for s in range(2):
            r[:, s * ncols:(s + 1) * ncols] = m[kp * 256 + s * 128:
                                                kp * 256 + (s + 1) * 128, :]
        out.append(r)
    return out


def _quant_weights(W, bias):
    """W [512, C] f32, bias [C] -> 6 DoubleRow fp8 arrays (a/b x W8,dW8,W8d)."""
    Ws = np.asarray(W, np.float32) * SW
    W8 = _f8(Ws)
    dW8 = _f8(Ws - W8.astype(np.float32))
    W8d = W8.copy()
    W8d[511, :] = _f8(bias * SX * SW / ALPHA)
    C = W.shape[1]
    a = {}
    a["W8a"], a["W8b"] = _dr_pack(W8, C)
    a["dW8a"], a["dW8b"] = _dr_pack(dW8, C)
    a["W8da"], a["W8db"] = _dr_pack(W8d, C)
    return a


def _make_in_maps(x, Wq, bq, Wk, bk, Wv, bv, Wg, bg):
    xf = np.asarray(x, np.float32).reshape(T, E)
    xs = xf * SX
    x8 = _f8(xs)
    dx8 = _f8(xs - x8.astype(np.float32))
    dx8p = dx8.copy()
    dx8p[:, 511] = np.float32(ALPHA)
    x8a, x8b = _dr_pack(np.ascontiguousarray(x8.T), T)
    dx8a, dx8b = _dr_pack(np.ascontiguousarray(dx8p.T), T)

    # e-major permutation for Wg columns within each head
    new_e, new_d = np.meshgrid(np.arange(D), np.arange(D), indexing="ij")
    old_of_new = (new_d * D + new_e).reshape(-1)

    in_maps = []
    for h in range(M):
        c0 = h * D
        g0 = h * D * D
        Wp = np.asarray(Wg, np.float32)[:, g0:g0 + D * D][:, old_of_new]
        bgp = np.asarray(bg, np.float32)[g0:g0 + D * D][old_of_new]
        gm = _quant_weights(Wp, bgp)

        QW = np.zeros((E, QVKW), np.float32)
        QW[:, 0:D] = Wq[:, c0:c0 + D]
        QW[:, D:2 * D] = Wv[:, c0:c0 + D]
        QW[:, 2 * D:3 * D] = Wk[:, c0:c0 + D]
        qb = np.zeros(QVKW, np.float32)
        qb[0:D] = bq[c0:c0 + D]
        qb[D:2 * D] = bv[c0:c0 + D]
        qb[2 * D:3 * D] = bk[c0:c0 + D]
        qm = _quant_weights(QW, qb)

        mp = dict(x8a=x8a, x8b=x8b, dx8a=dx8a, dx8b=dx8b)
        mp.update(gm)
        mp["Wq8a"], mp["Wq8b"] = qm["W8a"], qm["W8b"]
        mp["dWq8a"], mp["dWq8b"] = qm["dW8a"], qm["dW8b"]
        mp["Wq8da"], mp["Wq8db"] = qm["W8da"], qm["W8db"]
        in_maps.append(mp)
    return in_maps


def kernel(x, Wq, bq, Wk, bk, Wv, bv, Wg, bg):
    from concourse import bass_utils

    nc = _build()
    in_maps = _make_in_maps(x, Wq, bq, Wk, bk, Wv, bv, Wg, bg)
    res = bass_utils.run_bass_kernel_spmd(nc, in_maps, core_ids=list(range(M)))
    out = np.empty((B, N, H, D), dtype=np.float32)
    for h in range(M):
        oT = res.results[h]["outT"]           # [65, T], numerator SX*SW-scaled
        o = (oT[:D] / oT[D:D + 1]).T * ISCALE  # [T, 64]
        out[:, :, h, :] = o.reshape(B, N, D)
    return out.reshape(B, N, E)


# revision 17
# speedup vs baseline: 1.1509x; 1.0107x over previous
"""Fused OOQKV attention-with-generated-transform kernel for Trainium2.

Math (per head h):
  g = gelu(x @ Wg_h + bg_h)            # [T, 64, 64] per-token transform
  q,k,v = x @ W{q,k,v}_h + b           # [T, 64]
  qg[t] = q[t] @ g[t]
  att = softmax(qg @ k^T)              # per batch, no scaling
  out_h = att @ v

Sharding: head-parallel, 1 head per core (8 heads, 8 cores).

Implementation notes (v3):
- All projections are single-pass float16 matmuls (measured on HW: every
  matmul dtype streams 1 output column/cycle at 2.4 GHz, so f16's 11-bit
  mantissa is strictly better than bf16/f32r at identical cost; fp8
  DoubleRow only packs K=256 per instruction without halving per-column
  cost, so an error-compensated fp8 scheme loses to f16 1-pass).
- Biases enter as a K=1 ones-row matmul issued FIRST into each PSUM bank
  (start=True doubles as the bank reset).
- Wg columns are host-permuted e-major (col' = e*64+d) so gelu writes
  f16 contiguously and the qg d-reduction is innermost.
- qg on DVE: one f16 tensor_tensor multiply (4x/2x fast mode) against a
  broadcast q view, then a binary tree of f16 adds down to d=8 and a
  final tensor_reduce (tensor_reduce has no DVE fast mode, so the tree
  keeps most of the reduction in fast mode).
- Phase 2 runs after phase 1 (gelu and exp live in different ACT tables;
  interleaving them costs 1.3us per table reload). Within phase 2 the
  previous batch's AV matmuls are emitted mid-stream of the next batch's
  S matmuls, baseline-style, so PE/ACT pipeline across batches.
Host divides by the softmax denominator row (v is augmented with a ones
column initialized once on-chip) and transposes during the gather.
"""

import sys

sys.path.insert(0, "/opt/trn_rl_repo")

import numpy as np

B, N, E, H, D = 4, 1024, 512, 8, 64
T = B * N                 # 4096 flattened tokens
OC = 512                  # g output chunk (one PSUM bank)
NOC = (D * D) // OC       # 8 chunks per head
NKT = E // 128            # 4 contraction k-tiles
NTT = T // 128            # 32 token tiles
QVKW = 192                # fused q|v|k projection width
M = 8                     # cores
NMT = N // 128            # m tiles per batch
NNC = N // OC             # n chunks per batch

_cache = {}


def _build():
    if "nc" in _cache:
        return _cache["nc"]
    from contextlib import ExitStack

    import concourse.bass as bass
    import concourse.bacc as bacc
    import concourse.mybir as mybir
    import concourse.tile as tile
    from concourse.masks import make_identity

    F32 = mybir.dt.float32
    F32R = mybir.dt.float32r
    F16 = mybir.dt.float16
    AF = mybir.ActivationFunctionType
    ALU = mybir.AluOpType
    AX = mybir.AxisListType

    nc = bacc.Bacc(trn_type="TRN2")

    xT_d = nc.dram_tensor("xT16", [E, T], F16, kind="ExternalInput")
    Wg_d = nc.dram_tensor("Wg16", [E, D * D], F16, kind="ExternalInput")
    bg_d = nc.dram_tensor("bg16", [1, D * D], F16, kind="ExternalInput")
    Wq_d = nc.dram_tensor("Wqvk16", [E, QVKW], F16, kind="ExternalInput")
    bq_d = nc.dram_tensor("bqvk16", [1, QVKW], F16, kind="ExternalInput")
    outT_d = nc.dram_tensor("outT", [D + 1, T], F32, kind="ExternalOutput")

    with tile.TileContext(nc) as tc, ExitStack() as ctx:
        const = ctx.enter_context(tc.tile_pool(name="const", bufs=1))
        acts = ctx.enter_context(tc.tile_pool(name="acts", bufs=1))

        xT_sb, wg_sb, wq_sb = [], [], []
        for kt in range(NKT):
            xt = const.tile([128, T], F16, tag=f"x{kt}")
            nc.sync.dma_start(xt[:], xT_d[kt * 128:(kt + 1) * 128, :])
            xT_sb.append(xt)
            wt = const.tile([128, D * D], F16, tag=f"wg{kt}")
            nc.scalar.dma_start(wt[:], Wg_d[kt * 128:(kt + 1) * 128, :])
            wg_sb.append(wt)
            qt = const.tile([128, QVKW], F16, tag=f"wq{kt}")
            nc.scalar.dma_start(qt[:], Wq_d[kt * 128:(kt + 1) * 128, :])
            wq_sb.append(qt)
        bg_sb = const.tile([1, D * D], F16)
        nc.sync.dma_start(bg_sb[:], bg_d[:, :])
        bq_sb = const.tile([1, QVKW], F16)
        nc.sync.dma_start(bq_sb[:], bq_d[:, :])

        ones32 = const.tile([1, 128], F32)
        nc.gpsimd.memset(ones32[:], 1.0)
        ones16 = const.tile([1, 128], F16)
        nc.gpsimd.tensor_copy(ones16[:], ones32[:])
        onescol = const.tile([128, 1], F32)
        nc.gpsimd.memset(onescol[:], 1.0)
        ident = const.tile([128, 128], F16)
        make_identity(nc, ident[:])

        # persistent per-head activations
        v_sb = acts.tile([128, NTT, D + 1], F32R)   # v | ones column
        ov = onescol[:]
        ones_bc = bass.AP(tensor=ov.tensor, offset=ov.offset,
                          ap=[ov.ap[0], [0, NTT]])
        vv = v_sb[:]
        vcol = bass.AP(tensor=vv.tensor, offset=vv.offset + D,
                       ap=[vv.ap[0], [D + 1, NTT]])
        nc.vector.tensor_copy(vcol, ones_bc)
        kT_sb = acts.tile([D, T], F16)
        qgT_sb = acts.tile([D, T], F16)

        pmain = ctx.enter_context(
            tc.tile_pool(name="pmain", bufs=5, space="PSUM"))
        ptr = ctx.enter_context(
            tc.tile_pool(name="ptr", bufs=1, space="PSUM"))
        pav = ctx.enter_context(
            tc.tile_pool(name="pav", bufs=2, space="PSUM"))

        gpool = ctx.enter_context(tc.tile_pool(name="gp", bufs=2))
        dpool = ctx.enter_context(tc.tile_pool(name="dp", bufs=2))
        tpool = ctx.enter_context(tc.tile_pool(name="tp", bufs=1))
        espool = ctx.enter_context(tc.tile_pool(name="es", bufs=26))
        outp = ctx.enter_context(tc.tile_pool(name="outp", bufs=2))

        prev_tr = []      # (tc0, k_nat, qg_t) awaiting PE transpose

        def emit_transposes():
            for tc0_, kn, qt in prev_tr:
                for src, dst in ((kn, kT_sb), (qt, qgT_sb)):
                    p_t = ptr.tile([D, 128], F16, tag="tr", name="tr")
                    nc.tensor.transpose(p_t[:], src[:], ident[:])
                    nc.vector.tensor_copy(dst[:, tc0_:tc0_ + 128], p_t[:])
            prev_tr.clear()

        # ---------------- phase 1: projections, g, qg ----------------
        for tt in range(NTT):
            tc0 = tt * 128
            g_buf = gpool.tile([128, NOC * OC], F16, tag="g")
            pq = None
            q_t = k_nat = None

            pairs = [("qvk", 0), (1, 2), (3, 4), (5, 6), (7,)]
            for pair in pairs:
                members = []
                for m_ in pair:
                    if m_ == "qvk":
                        pq = pmain.tile([128, OC], F32, tag="pg", name="pq")
                        members.append((m_, pq[:, 0:QVKW],
                                        bq_sb[:], 0, QVKW))
                    else:
                        pg = pmain.tile([128, OC], F32, tag="pg",
                                        name=f"pg{m_}")
                        members.append((m_, pg[:], bg_sb[:, m_ * OC:
                                                         (m_ + 1) * OC],
                                        m_ * OC, OC))
                # bias first: start=True resets the bank
                for m_, o, brow, c0_, w_ in members:
                    nc.tensor.matmul(o, ones16[:], brow,
                                     start=True, stop=False)
                for kt in range(NKT):
                    for m_, o, brow, c0_, w_ in members:
                        rhs = (wq_sb[kt][:] if m_ == "qvk"
                               else wg_sb[kt][:, c0_:c0_ + w_])
                        nc.tensor.matmul(
                            o, xT_sb[kt][:, tc0:tc0 + 128], rhs,
                            start=False, stop=(kt == NKT - 1))

                for m_, o, brow, c0_, w_ in members:
                    if m_ == "qvk":
                        q_t = dpool.tile([128, D], F16, tag="q")
                        nc.vector.tensor_copy(q_t[:], pq[:, 0:D])
                        k_nat = dpool.tile([128, D], F16, tag="k")
                        nc.vector.tensor_copy(k_nat[:], pq[:, 2 * D:3 * D])
                        nc.vector.tensor_copy(v_sb[:, tt, 0:D],
                                              pq[:, D:2 * D])
                    else:
                        nc.scalar.activation(
                            g_buf[:, m_ * OC:(m_ + 1) * OC], o, AF.Gelu)

            # qg contraction on DVE (all f16, fast mode)
            prod = dpool.tile([128, NOC * OC], F16, tag="prod")
            gv = g_buf[:]
            g3 = bass.AP(tensor=gv.tensor, offset=gv.offset,
                         ap=[gv.ap[0], [D, D], [1, D]])
            qv = q_t[:]
            q3 = bass.AP(tensor=qv.tensor, offset=qv.offset,
                         ap=[qv.ap[0], [0, D], [1, D]])
            pv = prod[:]
            p3 = bass.AP(tensor=pv.tensor, offset=pv.offset,
                         ap=[pv.ap[0], [D, D], [1, D]])
            nc.vector.tensor_tensor(p3, g3, q3, op=ALU.mult)

            tre1 = tpool.tile([128, D * 32], F16, tag="t1")
            tre2 = tpool.tile([128, D * 16], F16, tag="t2")
            tre3 = tpool.tile([128, D * 8], F16, tag="t3")
            lvl_in, w_ = prod, 64
            for tre in (tre1, tre2, tre3):
                h_ = w_ // 2
                iv = lvl_in[:]
                a0 = bass.AP(tensor=iv.tensor, offset=iv.offset,
                             ap=[iv.ap[0], [w_, D], [1, h_]])
                a1 = bass.AP(tensor=iv.tensor, offset=iv.offset + h_,
                             ap=[iv.ap[0], [w_, D], [1, h_]])
                ov_ = tre[:]
                o3 = bass.AP(tensor=ov_.tensor, offset=ov_.offset,
                             ap=[ov_.ap[0], [h_, D], [1, h_]])
                nc.vector.tensor_tensor(o3, a0, a1, op=ALU.add)
                lvl_in, w_ = tre, h_
            qg_t = dpool.tile([128, D], F16, tag="qg")
            t3v = tre3[:]
            t33 = bass.AP(tensor=t3v.tensor, offset=t3v.offset,
                          ap=[t3v.ap[0], [8, D], [1, 8]])
            with nc.allow_low_precision(reason="f16 qg partials"):
                nc.vector.tensor_reduce(qg_t[:], t33, axis=AX.X, op=ALU.add)

            emit_transposes()
            prev_tr.append((tc0, k_nat, qg_t))
        emit_transposes()

        # ---------------- phase 2: attention ----------------
        es = {}
        pending_av = []   # (b, nch, es-dict) awaiting AV emission

        def emit_av():
            if not pending_av:
                return
            for bb, nch, esd in pending_av:
                pv_ = pav.tile([D + 1, OC], F32, tag="av", name=f"pav{nch}")
                for mt in range(NMT):
                    nc.tensor.matmul(pv_[:], v_sb[:, bb * NMT + mt, :],
                                     esd[mt][:],
                                     start=(mt == 0), stop=(mt == NMT - 1))
                nc0 = bb * N + nch * OC
                o_t = outp.tile([D + 1, OC], F32, tag="o", name="o_t")
                nc.vector.tensor_copy(o_t[:], pv_[:])
                nc.sync.dma_start(outT_d[:, nc0:nc0 + OC], o_t[:])
            pending_av.clear()

        for b in range(B):
            for mt in range(NMT):
                if mt == 2:
                    emit_av()     # prior batch's AV, mid-stream
                mc0 = b * N + mt * 128
                for nch in range(NNC):
                    nc0 = b * N + nch * OC
                    ps_ = pmain.tile([128, OC], F32, tag="pg", name="ps")
                    nc.tensor.matmul(ps_[:], kT_sb[:, mc0:mc0 + 128],
                                     qgT_sb[:, nc0:nc0 + OC],
                                     start=True, stop=True)
                    e_t = espool.tile([128, OC], F32R, tag="es")
                    nc.scalar.activation(e_t[:], ps_[:], AF.Exp)
                    es[(mt, nch)] = e_t
            for nch in range(NNC):
                pending_av.append((b, nch, {mt: es[(mt, nch)]
                                            for mt in range(NMT)}))
        emit_av()

    nc.compile()
    _cache["nc"] = nc
    return nc


def _make_in_maps(x, Wq, bq, Wk, bk, Wv, bv, Wg, bg):
    import ml_dtypes  # noqa: F401
    F16 = np.float16
    xT16 = np.ascontiguousarray(
        np.asarray(x, np.float32).reshape(T, E).T).astype(F16)

    # e-major permutation for Wg columns within each head
    new_e, new_d = np.meshgrid(np.arange(D), np.arange(D), indexing="ij")
    old_of_new = (new_d * D + new_e).reshape(-1)

    in_maps = []
    for h in range(M):
        c0 = h * D
        g0 = h * D * D
        Wp = np.asarray(Wg, np.float32)[:, g0:g0 + D * D][:, old_of_new]
        bgp = np.asarray(bg, np.float32)[g0:g0 + D * D][old_of_new]

        QW = np.zeros((E, QVKW), np.float32)
        QW[:, 0:D] = Wq[:, c0:c0 + D]
        QW[:, D:2 * D] = Wv[:, c0:c0 + D]
        QW[:, 2 * D:3 * D] = Wk[:, c0:c0 + D]
        qb = np.zeros((1, QVKW), np.float32)
        qb[0, 0:D] = bq[c0:c0 + D]
        qb[0, D:2 * D] = bv[c0:c0 + D]
        qb[0, 2 * D:3 * D] = bk[c0:c0 + D]

        in_maps.append(dict(
            xT16=xT16,
            Wg16=np.ascontiguousarray(Wp).astype(F16),
            bg16=bgp.reshape(1, D * D).astype(F16),
            Wqvk16=np.ascontiguousarray(QW).astype(F16),
            bqvk16=qb.astype(F16),
        ))
    return in_maps


def kernel(x, Wq, bq, Wk, bk, Wv, bv, Wg, bg):
    from concourse import bass_utils

    nc = _build()
    in_maps = _make_in_maps(x, Wq, bq, Wk, bk, Wv, bv, Wg, bg)
    res = bass_utils.run_bass_kernel_spmd(nc, in_maps, core_ids=list(range(M)))
    out = np.empty((B, N, H, D), dtype=np.float32)
    for h in range(M):
        oT = res.results[h]["outT"]           # [65, T]
        o = (oT[:D] / oT[D:D + 1]).T          # [T, 64]
        out[:, :, h, :] = o.reshape(B, N, D)
    return out.reshape(B, N, E)
